# revision 9
# baseline (speedup 1.0000x reference)
"""BitLinear (activation int8-quant + ternary weight) + squared-ReLU on 8 Trainium2
NeuronCores.

Sharding: tensor-parallel over weight rows (out_features). Each core receives the
full activation tensor and a 1/8 slice of the weight matrix, computes its slice of
the GEMM + squared ReLU, and the host concatenates the slices.

The global weight scale mean(|W|) is computed on-device: per-core partial sums +
a scalar AllReduce across the 8 cores.

Math notes:
  - x_q = round(x * 127/scale) with scale = clip(amax_row(|x|), 1e-5). Values are
    integers in [-127, 127] -> exactly representable in bf16 (8 significand bits).
  - w_q in {-1, 0, 1} = (w > 0.5*ws) - (w < -0.5*ws) via exact fp32 compares.
  - The bf16 GEMM with fp32 PSUM accumulation is exact (all products are small
    integers, partial sums < 2^24).
  - x rounding reproduces fp32 round-to-nearest-even (jnp.round semantics) with
    the +1.5*2^23 magic-constant trick, applied after the product is rounded to
    fp32 (same double-rounding as the reference).
  - squared ReLU with the dequant scale folded in:
    out = Square(Relu((w_scale/scale) * psum)).
"""

import sys

if "/opt/trn_rl_repo" not in sys.path:
    sys.path.insert(0, "/opt/trn_rl_repo")

import numpy as np

import concourse.bacc as bacc
import concourse.bass_isa as bass_isa
import concourse.mybir as mybir
import concourse.tile as tile
from concourse.bass_utils import run_bass_kernel_spmd
from concourse.masks import make_identity

dt = mybir.dt
NCORES = 8
C_MAGIC = 1.5 * 2**23  # fp32 round-to-nearest-even forcing constant

# Stash of the most recent BassKernelResults (test harness reads exec_time_ns).
LAST_RESULTS = None

_NC_CACHE = {}


def _build(T, K, O, max_val):
    """Build + compile the per-core Bass module.

    Per-core tensors: x [T, K] f32 (replicated), w [O, K] f32 (this core's rows),
    out [T, O] f32.
    """
    assert T % 128 == 0 and K % 256 == 0 and O % 512 == 0
    TT = T // 128     # token tiles
    KT = K // 128     # contraction tiles
    OC = O // 512     # psum-width output chunks per core
    OT = O // 128     # weight row tiles
    n_w_elem = float(NCORES * O * K)

    nc = bacc.Bacc("TRN2", target_bir_lowering=False, debug=False,
                   num_devices=NCORES)

    x_ap = nc.dram_tensor("x", [T, K], dt.float32, kind="ExternalInput").ap()
    w_ap = nc.dram_tensor("w", [O, K], dt.float32, kind="ExternalInput").ap()
    out_ap = nc.dram_tensor("out", [T, O], dt.float32, kind="ExternalOutput").ap()

    with tile.TileContext(nc) as tc:
        with (
            tc.tile_pool(name="const", bufs=1) as const_pool,
            tc.tile_pool(name="wres", bufs=1) as wres_pool,
            tc.tile_pool(name="xs", bufs=3) as x_pool,
            tc.tile_pool(name="xq", bufs=3) as xq_pool,
            tc.tile_pool(name="osb", bufs=3) as out_pool,
            tc.tile_pool(name="sc", bufs=8) as sc_pool,
            tc.tile_pool(name="tps", bufs=4, space="PSUM") as tps_pool,
            tc.tile_pool(name="mmps", bufs=2, space="PSUM") as mm_pool,
            tc.tile_pool(name="dram", bufs=2, space="DRAM") as dram_pool,
        ):
            ident = const_pool.tile([128, 128], dt.bfloat16)
            make_identity(nc, ident[:])

            # ------------- weight phase (staging pools freed after) -------------
            wqT = wres_pool.tile([128, KT * O], dt.bfloat16)
            ws = wres_pool.tile([128, 1], dt.float32)          # w_scale
            with (
                tc.tile_pool(name="wstage", bufs=2) as wstage_pool,
                tc.tile_pool(name="wq", bufs=1) as wq_pool,
            ):
                # pass 1: stream w tiles, two-stage |w| partial sums
                wpart = wres_pool.tile([128, OT * KT], dt.float32)
                for r in range(OT):
                    wt = wstage_pool.tile([128, K], dt.float32, tag="wstage")
                    nc.sync.dma_start(wt[:], w_ap[128 * r:128 * (r + 1), :])
                    nc.vector.tensor_reduce(
                        wpart[:, KT * r:KT * (r + 1)],
                        wt[:].rearrange("p (a b) -> p a b", b=128),
                        axis=mybir.AxisListType.X,
                        op=mybir.AluOpType.add, apply_absolute_value=True)
                wpart1 = wres_pool.tile([128, 1], dt.float32)
                nc.vector.tensor_reduce(wpart1[:], wpart[:],
                                        axis=mybir.AxisListType.X,
                                        op=mybir.AluOpType.add)
                wtot = wres_pool.tile([128, 1], dt.float32)
                nc.gpsimd.partition_all_reduce(wtot[:], wpart1[:], channels=128,
                                               reduce_op=bass_isa.ReduceOp.add)
                # scalar AllReduce across the 8 cores via DRAM bounce buffers
                cc_in = dram_pool.tile([1, 1], dt.float32)
                cc_out = dram_pool.tile([1, 1], dt.float32)
                nc.gpsimd.dma_start(cc_in[:], wtot[0:1, 0:1])
                nc.gpsimd.collective_compute(
                    "AllReduce", mybir.AluOpType.add,
                    replica_groups=[list(range(NCORES))],
                    ins=[cc_in.opt()], outs=[cc_out.opt()])
                wsum_bc = wres_pool.tile([128, 1], dt.float32)
                nc.gpsimd.dma_start(wsum_bc[:], cc_out[:].broadcast_to([128, 1]))

                nc.vector.tensor_scalar_mul(ws[:], wsum_bc[:], 1.0 / n_w_elem)
                halfws = wres_pool.tile([128, 1], dt.float32)  # +0.5 * w_scale
                nc.vector.tensor_scalar_mul(halfws[:], ws[:], 0.5)
                neghws = wres_pool.tile([128, 1], dt.float32)  # -0.5 * w_scale
                nc.vector.tensor_scalar_mul(neghws[:], ws[:], -0.5)

                # pass 2: re-stream w, exact strict-compare ternarization
                wq_tiles = []
                for r in range(OT):
                    wt = wstage_pool.tile([128, K], dt.float32, tag="wstage")
                    nc.sync.dma_start(wt[:], w_ap[128 * r:128 * (r + 1), :])
                    tp = wstage_pool.tile([128, K], dt.float32, tag="wquant")
                    nc.vector.tensor_scalar(tp[:], wt[:], halfws[:], None,
                                            op0=mybir.AluOpType.is_gt)
                    tn = wstage_pool.tile([128, K], dt.float32, tag="wquant2")
                    nc.vector.tensor_scalar(tn[:], wt[:], neghws[:], None,
                                            op0=mybir.AluOpType.is_lt)
                    wq = wq_pool.tile([128, K], dt.bfloat16, tag=f"wq{r}")
                    nc.vector.tensor_tensor(wq[:], tp[:], tn[:],
                                            op=mybir.AluOpType.subtract)
                    wq_tiles.append(wq)

                # transpose wq -> wqT: column block j holds k-tile j x all O
                for j in range(KT):
                    for h in range(OT // 4):
                        ps = tps_pool.tile([128, 512], dt.bfloat16, tag="tps")
                        for q in range(4):
                            r = 4 * h + q
                            nc.tensor.transpose(
                                ps[:, 128 * q:128 * (q + 1)],
                                wq_tiles[r][:, 128 * j:128 * (j + 1)], ident[:])
                        dst = wqT[:, O * j + 512 * h:O * j + 512 * (h + 1)]
                        if h % 2 == 0:
                            nc.scalar.copy(dst, ps[:])
                        else:
                            nc.vector.tensor_copy(dst, ps[:])

            # ---------------- main loop over token tiles ----------------
            with tc.tile_pool(name="xqt", bufs=12) as xqt_pool:
                for t in range(TT):
                    xt = x_pool.tile([128, K], dt.float32, tag="x")
                    nc.sync.dma_start(xt[:], x_ap[128 * t:128 * (t + 1), :])

                    amax = sc_pool.tile([128, 1], dt.float32, tag="amax")
                    nc.vector.tensor_reduce(amax[:], xt[:],
                                            axis=mybir.AxisListType.X,
                                            op=mybir.AluOpType.max,
                                            apply_absolute_value=True)
                    nc.vector.tensor_scalar_max(amax[:], amax[:], 1e-5)
                    rinv = sc_pool.tile([128, 1], dt.float32, tag="rinv")
                    nc.vector.reciprocal(rinv[:], amax[:])
                    rs = sc_pool.tile([128, 1], dt.float32, tag="rs")
                    nc.vector.tensor_scalar_mul(rs[:], rinv[:], float(max_val))
                    g = sc_pool.tile([128, 1], dt.float32, tag="g")
                    nc.vector.tensor_tensor(g[:], ws[:], rinv[:],
                                            op=mybir.AluOpType.mult)

                    # x_q = rint(fl(x * rs)): fp32 product on ACT, then RNE to
                    # integer via +C/-C on DVE, cast to exact bf16 integers
                    xqf = xq_pool.tile([128, K], dt.float32, tag="xqf")
                    nc.scalar.activation(xqf[:], xt[:],
                                         mybir.ActivationFunctionType.Copy,
                                         scale=rs[:])
                    xq = xq_pool.tile([128, K], dt.bfloat16, tag="xq")
                    nc.vector.tensor_scalar(xq[:], xqf[:], C_MAGIC, C_MAGIC,
                                            op0=mybir.AluOpType.add,
                                            op1=mybir.AluOpType.subtract)

                    # transpose xq -> xqT [128, KT*128] bf16 (k on partitions)
                    xqT = xqt_pool.tile([128, KT * 128], dt.bfloat16, tag="xqT")
                    half = KT // 2
                    for hh in range(2):
                        ps = tps_pool.tile([128, half * 128], dt.bfloat16,
                                           tag="tps")
                        for j in range(half):
                            jj = hh * half + j
                            nc.tensor.transpose(
                                ps[:, 128 * j:128 * (j + 1)],
                                xq[:, 128 * jj:128 * (jj + 1)], ident[:])
                        dst = xqT[:, 128 * half * hh:128 * half * (hh + 1)]
                        if hh == 0:
                            nc.scalar.copy(dst, ps[:])
                        else:
                            nc.vector.tensor_copy(dst, ps[:])

                    # GEMM: psum[t, o] += xqT[k, t].T @ wqT[k, o]
                    psums = [mm_pool.tile([128, 512], dt.float32, tag=f"mm{c}",
                                          name=f"mm{c}")
                             for c in range(OC)]
                    for j in range(KT):
                        lhsT = xqT[:, 128 * j:128 * (j + 1)]
                        for c in range(OC):
                            nc.tensor.matmul(
                                psums[c][:], lhsT,
                                wqT[:, O * j + 512 * c:O * j + 512 * (c + 1)],
                                start=(j == 0), stop=(j == KT - 1))

                    # out = Square(Relu(g * psum))
                    osb = out_pool.tile([128, O], dt.float32, tag="osb")
                    for c in range(OC):
                        nc.scalar.activation(osb[:, 512 * c:512 * (c + 1)],
                                             psums[c][:],
                                             mybir.ActivationFunctionType.Relu,
                                             scale=g[:])
                    sq = out_pool.tile([128, O], dt.float32, tag="sq")
                    nc.scalar.activation(sq[:], osb[:],
                                         mybir.ActivationFunctionType.Square)
                    nc.sync.dma_start(out_ap[128 * t:128 * (t + 1), :], sq[:])

    nc.compile()
    return nc


def _get_nc(T, K, O, max_val):
    key = (T, K, O, max_val)
    if key not in _NC_CACHE:
        _NC_CACHE[key] = _build(T, K, O, max_val)
    return _NC_CACHE[key]


def kernel(x, weight, bits=8):
    global LAST_RESULTS
    x = np.asarray(x, dtype=np.float32)
    weight = np.asarray(weight, dtype=np.float32)
    bits = int(bits)
    max_val = (1 << (bits - 1)) - 1

    lead_shape = x.shape[:-1]
    K = x.shape[-1]
    T = int(np.prod(lead_shape))
    O_total, K_w = weight.shape
    assert K == K_w and O_total % NCORES == 0
    O = O_total // NCORES

    nc = _get_nc(T, K, O, max_val)

    x2 = np.ascontiguousarray(x.reshape(T, K))
    in_maps = [{"x": x2, "w": np.ascontiguousarray(weight[i * O:(i + 1) * O])}
               for i in range(NCORES)]
    res = run_bass_kernel_spmd(nc, in_maps, list(range(NCORES)))
    LAST_RESULTS = res

    out = np.concatenate([res.results[i]["out"] for i in range(NCORES)], axis=1)
    return out.reshape(*lead_shape, O_total)
